# revision 18
# baseline (speedup 1.0000x reference)
"""Trainium2 Bass kernel for nn_BilinearAttnPool (B=32, C=2048, H=24, W=12, M=8).

Math notes (why this is exact enough):
  reference: attn = relu(BN(conv1x1(f)))  (attn >= 0)
             x = clip(f * attn, min=1e-6) ** 3 ; pooled = mean_hw(x) ** (1/3)
  Since attn >= 0:  clip(f*attn, eps) = attn * relu(f)  except where f*attn <= eps,
  where the reference contributes eps^3 = 1e-18 (utterly negligible).  Therefore
      pooled(b,m,c)^3 ∝ S(b,m,c) = sum_hw attn(b,m,hw)^3 * relu(f(b,c,hw))^3  -- a matmul.
  The 1/HW mean factor is a global scale that cancels in the final L2 normalize,
  as does any global constant through the ^(1/3) and sign-sqrt (monotone powers).
  pooled >= 0 always => sign-sqrt == sqrt;  z = S^(1/6) = exp(ln(S)/6), and the
  final L2 normalize folds into the exp bias:
      out = exp(ln(S)/6 - ln(n)/2),   n = sum_{m,c} S^(1/3)   (per sample).

Layout notes (v3):
  The host supplies the features TWICE (less total HBM bytes than one fp32 copy):
    fc[b, p, i, hw] = 8 * f[b, 16p+i, hw]  in fp8 e3m4 -- conv matmul rhs; the
       conv weights are pre-scaled x64 and the 1/128 is folded into the
       BN+relu activation's input scale.
    ft tiles [9, 128, C] bf16             -- hw-on-partitions, pooling side:
       tile t=2b+k (k=0,1): partition p = pixel 128k+p of sample b
       tile 8: partition 32b+j = tail pixel 256+j of sample b (packed 4x32)
  relu(f)^3 is computed tile-wise straight in the [hw, c] layout (ACT square,
  DVE max, DVE mul) and feeds the pooled matmul rhs directly with natural
  c-order columns.  attn^3 is transposed with tiny PE transposes; the hw-tail
  transpose lands at partition offset 32b via tile_position=(32b,32b) so the
  K=32 tail matmuls of all 4 samples pack into disjoint 32x32 array tiles.
  Dummy matmuls into a spare PSUM bank warm the PE HAM clock gate during the
  initial load phase.  Const loads ride SWDGE (gpsimd); feature loads are
  split across both HWDGE issuers (sync + scalar) to halve issue serialization.

Sharding: pure data parallel, batch 32 -> 8 cores x 4 samples.
"""

import numpy as np
import ml_dtypes

B, C, H, W, M = 32, 2048, 24, 12, 8
NCORES = 8
BL = B // NCORES          # 4 samples per core
HW = H * W                # 288
P = 128
CI = C // P               # 16 channels per partition
NT = 2 * BL + 1           # 9 hw-layout tiles
NWARM = 30                # PE HAM warm-up dummy matmuls
TORD = [0, 1, 2, 3, 8, 4, 5, 6, 7]   # ft DRAM arrival order
FSCALE = 2.0              # fp8 feature pre-scale
WSCALE = 64.0             # fp8 conv-weight pre-scale
BN_EPS = 1e-3

_CACHE = {}


def _build_program():
    import concourse.bass as bass
    import concourse.tile as tile
    import concourse.mybir as mybir
    from concourse import bacc

    # Pin every ACT function to the one table set that contains all of
    # Square/Relu/Ln/Exp/Copy, so the whole kernel does a single
    # ACT_TABLE_LOAD instead of ping-ponging between sets (~1.3us each).
    import concourse.bacc as bacc_mod
    _orig_tables = bacc_mod.get_activation_tables

    def _pinned_tables(arch):
        tabs = dict(_orig_tables(arch))
        if "natural_log_exp_and_others" in tabs:
            for k in tabs:
                if k != "natural_log_exp_and_others":
                    tabs[k] = set()
        return tabs

    bacc_mod.get_activation_tables = _pinned_tables
    try:
        nc = _build_inner(bacc_mod, tile, mybir)
    finally:
        bacc_mod.get_activation_tables = _orig_tables
    return nc


def _build_inner(bacc, tile, mybir):
    dt = mybir.dt
    AF = mybir.ActivationFunctionType

    nc = bacc.Bacc("TRN2", target_bir_lowering=False, debug=False,
                   num_devices=NCORES)

    fc_d = nc.declare_dram_parameter("fc", [BL, P, CI * HW], dt.float8e3, isOutput=False)
    ft_d = nc.declare_dram_parameter("ft", [NT, P, C], dt.bfloat16, isOutput=False)
    w2t_d = nc.declare_dram_parameter("w2t", [P, CI * M], dt.float8e3, isOutput=False)
    dvec_d = nc.declare_dram_parameter("dvec", [P, 1], dt.float32, isOutput=False)
    gmat3_d = nc.declare_dram_parameter("gmat3", [P, P], dt.float32, isOutput=False)
    ident_d = nc.declare_dram_parameter("ident", [P, 32], dt.bfloat16, isOutput=False)
    out_d = nc.declare_dram_parameter("out", [BL, M, C], dt.float32, isOutput=True)

    with tile.TileContext(nc) as tc:
        with (
            tc.tile_pool(name="const", bufs=1) as cpool,
            tc.tile_pool(name="persist", bufs=1) as perst,
            tc.tile_pool(name="ftp", bufs=1) as ftp,
            tc.tile_pool(name="fcp", bufs=1) as fcp,
            tc.tile_pool(name="sqp", bufs=3) as sqp,
            tc.tile_pool(name="rlp", bufs=3) as rlp,
            tc.tile_pool(name="f3p", bufs=1) as f3p,
            tc.tile_pool(name="psa", bufs=1, space="PSUM") as psa_pool,
            tc.tile_pool(name="psp", bufs=1, space="PSUM") as psp_pool,
            tc.tile_pool(name="psn", bufs=1, space="PSUM") as psn_pool,
        ):
            w2t = cpool.tile([P, CI * M], dt.float8e3)
            dvec = cpool.tile([P, 1], dt.float32)
            gmat3 = cpool.tile([P, P], dt.float32)
            ident = cpool.tile([P, 32], dt.bfloat16)
            dum = cpool.tile([P, 512], dt.bfloat16)
            nc.sync.dma_start(w2t[:], w2t_d.ap())
            nc.sync.dma_start(dvec[:], dvec_d.ap())
            nc.sync.dma_start(ident[:], ident_d.ap())
            nc.sync.dma_start(gmat3[:], gmat3_d.ap())

            ftb = [None] * NT
            fcb = [None] * BL

            def load_ft_group(slots):
                # one SWDGE DMA covering consecutive DRAM ft slots
                i0 = min(slots)
                grp = ftp.tile([P, len(slots), C], dt.bfloat16,
                               name=f"ftg{i0}", tag=f"ftg{i0}")
                for k, i in enumerate(slots):
                    ftb[TORD[i]] = grp[:, k, :]
                nc.gpsimd.dma_start(
                    grp[:],
                    ft_d.ap()[i0:i0 + len(slots)].rearrange("t p c -> p t c"))

            def load_fc(b0, n):
                grp = fcp.tile([P, n, CI, HW], dt.float8e3, name=f"fcg{b0}",
                               tag=f"fcg{b0}")
                for k in range(n):
                    fcb[b0 + k] = grp[:, k, :, :]
                nc.gpsimd.dma_start(
                    grp[:],
                    fc_d.ap()[b0:b0 + n].rearrange("b p (i hw) -> p b i hw",
                                                   i=CI))

            # all feature bytes ride SWDGE (Q0) -- the empirically fastest
            # solo ring -- fine groups during the ring ramp, then perfectly
            # alternating fc/ft groups so no engine sees bunched arrivals
            load_fc(0, 1)
            load_ft_group([0])
            load_ft_group([1])
            load_fc(1, 1)
            load_ft_group([2, 3])
            load_fc(2, 1)
            load_ft_group([4])
            load_ft_group([5, 6])
            load_fc(3, 1)
            load_ft_group([7, 8])

            attn = perst.tile([P, HW], dt.bfloat16)
            sqt = perst.tile([P, HW], dt.bfloat16)
            a3 = perst.tile([P, HW], dt.bfloat16)
            a3t = [perst.tile([P, 2, 32], dt.bfloat16, name=f"a3t{b}",
                              tag=f"a3t{b}") for b in range(BL)]
            a3tail = perst.tile([P, 32], dt.bfloat16)

            psA = psa_pool.tile([P, HW], dt.float32)
            psPl = psp_pool.tile([P, C // 2], dt.float32, tag="pspl")
            psPh = psp_pool.tile([P, C // 2], dt.float32, tag="psph")
            psPx = [psPl, psPh]
            psT3 = psn_pool.tile([P, 3, 32], dt.bfloat16, tag="pst")
            psT = psT3[:, 0:2, :]
            psTt = psT3[:, 2, :]
            psB4 = psn_pool.tile([P, BL], dt.float32, tag="psb4")
            psD = psn_pool.tile([P, P], dt.float32, tag="psdum")

            nc.vector.memset(dum[:], 0.0)

            # HAM warm-up: cheap dummies flip the PE clock gate to 2.4 GHz,
            # then 16 now-warm zero-matmuls cover all of psP (keeps ln()
            # finite on unused rows, replacing a 1.8us DVE memset)
            for _ in range(NWARM):
                nc.tensor.matmul(psD[0:M, 0:P], dum[:, 0:M], dum[:, 0:P],
                                 start=True, stop=True, skip_group_check=True)
            for jj in range(4):
                for rr in range(4):
                    nc.tensor.matmul(
                        psPx[jj // 2][32 * rr:32 * rr + 32,
                                      512 * (jj % 2):512 * (jj % 2 + 1)],
                        dum[:, 0:32], dum[:],
                        start=True, stop=True,
                        tile_position=(0, 32 * rr),
                        skip_group_check=True)

            f3 = [None] * NT

            def elementwise(t, sq_dve=False, rl_gps=False):
                # f3 = relu(f)^3 = f^2 * max(f, 0), all bf16 [128, 2048]
                sq = sqp.tile([P, C], dt.bfloat16)
                rl = rlp.tile([P, C], dt.bfloat16)
                f3[t] = f3p.tile([P, C], dt.bfloat16, name=f"f3_{t}", tag=f"f3_{t}")
                if sq_dve:
                    nc.vector.tensor_mul(sq[:], ftb[t][:], ftb[t][:])
                else:
                    nc.scalar.activation(sq[:], ftb[t][:], AF.Square)
                if rl_gps:
                    nc.gpsimd.tensor_scalar_max(rl[:], ftb[t][:], 0.0)
                else:
                    nc.vector.tensor_scalar_max(rl[:], ftb[t][:], 0.0)
                nc.vector.tensor_mul(f3[t][:], sq[:], rl[:])

            def elementwise67():
                # the last-arriving pair (tiles 6, 7) lands together: split
                # the two squares across ACT/DVE and order the DVE stream so
                # neither mul head-of-line blocks the other
                sq6 = sqp.tile([P, C], dt.bfloat16)
                sq7 = sqp.tile([P, C], dt.bfloat16)
                rl6 = rlp.tile([P, C], dt.bfloat16)
                rl7 = rlp.tile([P, C], dt.bfloat16)
                f3[6] = f3p.tile([P, C], dt.bfloat16, name="f3_6", tag="f3_6")
                f3[7] = f3p.tile([P, C], dt.bfloat16, name="f3_7", tag="f3_7")
                nc.scalar.activation(sq6[:], ftb[6][:], AF.Square)
                nc.scalar.activation(sq7[:], ftb[7][:], AF.Square)
                nc.vector.tensor_scalar_max(rl6[:], ftb[6][:], 0.0)
                nc.vector.tensor_scalar_max(rl7[:], ftb[7][:], 0.0)
                nc.vector.tensor_mul(f3[6][:], sq6[:], rl6[:])
                nc.vector.tensor_mul(f3[7][:], sq7[:], rl7[:])

            def conv(b):
                for i in range(CI):
                    nc.tensor.matmul(
                        psA[32 * b:32 * b + M, :],
                        w2t[:, M * i:M * (i + 1)],
                        fcb[b][:, i, :],
                        start=(i == 0), stop=(i == CI - 1),
                        tile_position=(0, 32 * b),
                        skip_group_check=True,
                    )

            def attn_cube(b):
                # attn = relu(conv/512 + d); a3 = attn^3; transpose to [hw, m]
                rs = slice(32 * b, 32 * b + M)
                ts = slice(32 * b, 32 * b + 32)
                nc.scalar.activation(attn[rs, :], psA[rs, :], AF.Relu,
                                     bias=dvec[rs, :],
                                     scale=1.0 / (FSCALE * WSCALE))
                nc.scalar.activation(sqt[rs, :], attn[rs, :], AF.Square)
                nc.vector.tensor_mul(a3[rs, :], sqt[rs, :], attn[rs, :])
                for k in range(2):
                    nc.tensor.transpose(
                        psT[:, k, :], a3[ts, P * k:P * (k + 1)], ident[ts, :],
                        tile_position=(32 * b, 0))
                nc.tensor.transpose(
                    psTt[ts], a3[ts, 256:HW], ident[ts, :],
                    tile_position=(32 * b, 32 * b))
                nc.vector.tensor_copy(a3t[b][:], psT[:])
                nc.vector.tensor_copy(a3tail[ts, :], psTt[ts])

            def pooled(b):
                # psP[32b+m, c] = sum_hw a3t[hw, m] * f3[hw, c]
                rs = slice(32 * b, 32 * b + M)
                ts = slice(32 * b, 32 * b + 32)
                def mm_k(k, j, start):
                    ps = psPx[j // 2]
                    pcs = slice(512 * (j % 2), 512 * (j % 2 + 1))
                    cs = slice(512 * j, 512 * (j + 1))
                    nc.tensor.matmul(
                        ps[rs, pcs], a3t[b][:, k, 0:M], f3[2 * b + k][:, cs],
                        start=start, stop=False,
                        tile_position=(0, 32 * b),
                        skip_group_check=True)
                def mm_tail(j):
                    ps = psPx[j // 2]
                    pcs = slice(512 * (j % 2), 512 * (j % 2 + 1))
                    cs = slice(512 * j, 512 * (j + 1))
                    nc.tensor.matmul(
                        ps[rs, pcs], a3tail[ts, 0:M], f3[8][ts, cs],
                        start=False, stop=True,
                        tile_position=(32 * b, 32 * b),
                        skip_group_check=True)
                if b < BL - 1:
                    for k in range(2):
                        for j in range(4):
                            mm_k(k, j, start=(k == 0))
                else:
                    # last sample: finish psP chunk-by-chunk so the Ln/Exp
                    # post chunks overlap the remaining matmuls
                    for j in range(4):
                        mm_k(0, j, start=True)
                        mm_k(1, j, start=False)
                        mm_tail(j)

            def pooled_tails(b):
                rs = slice(32 * b, 32 * b + M)
                ts = slice(32 * b, 32 * b + 32)
                for j in range(4):
                    ps = psPx[j // 2]
                    pcs = slice(512 * (j % 2), 512 * (j % 2 + 1))
                    cs = slice(512 * j, 512 * (j + 1))
                    nc.tensor.matmul(
                        ps[rs, pcs], a3tail[ts, 0:M], f3[8][ts, cs],
                        start=False, stop=True,
                        tile_position=(32 * b, 32 * b),
                        skip_group_check=True)

            # software-pipelined emission: conv one sample ahead of the
            # attn/pooled chain so the PE stream never head-of-line blocks
            conv(0)
            elementwise(0)
            elementwise(1)
            conv(1)
            attn_cube(0)
            pooled(0)
            elementwise(2)
            elementwise(8)
            elementwise(3)
            conv(2)
            attn_cube(1)
            pooled(1)
            elementwise(4, sq_dve=True)
            elementwise(5, sq_dve=True)
            conv(3)
            attn_cube(2)
            pooled(2)
            pooled_tails(0)
            pooled_tails(1)
            pooled_tails(2)
            attn_cube(3)
            elementwise67()
            pooled(3)

            # post: z = S^(1/6), n = sum_c S^(1/3), out = exp(ln(S)/6 - ln(n)/2)
            # Ln and the S^(1/3) row-accumulate run per 512-col chunk so they
            # overlap the final sample's pooled matmuls.
            lns = perst.tile([P, C], dt.float32)
            wsb = perst.tile([P, C], dt.bfloat16)
            part = perst.tile([P, 2], dt.float32)
            nsum = perst.tile([P, 1], dt.float32)
            lnn = perst.tile([P, 1], dt.float32)
            nbias = perst.tile([P, 1], dt.float32)
            fm = perst.tile([P, C], dt.float32)

            for j in range(2):
                cs = slice(1024 * j, 1024 * (j + 1))
                nc.scalar.activation(lns[:, cs], psPx[j][:], AF.Ln)
                nc.scalar.activation(wsb[:, cs], lns[:, cs], AF.Exp,
                                     scale=1.0 / 3.0,
                                     accum_out=part[:, j:j + 1])
            nc.tensor.matmul(psB4[:, 0:2], gmat3[:], part[:])  # group sums
            scr2 = perst.tile([P, 2], dt.float32)
            nc.scalar.activation(scr2[:], psB4[:, 0:2], AF.Copy,
                                 accum_out=nsum[:])
            nc.scalar.activation(lnn[:], nsum[:], AF.Ln)
            nc.vector.tensor_scalar_mul(nbias[:], lnn[:], -0.5)
            nc.scalar.activation(fm[:], lns[:], AF.Exp, scale=1.0 / 6.0,
                                 bias=nbias[:])

            nc.sync.dma_start(out_d.ap()[0], fm[0:M, :])
            nc.scalar.dma_start(out_d.ap()[1], fm[32:32 + M, :])
            nc.sync.dma_start(out_d.ap()[2], fm[64:64 + M, :])
            nc.scalar.dma_start(out_d.ap()[3], fm[96:96 + M, :])

    nc.compile()
    return nc


def _host_prep(conv_w, bn_scale, bn_bias, bn_mean, bn_var):
    g = (bn_scale / np.sqrt(bn_var + BN_EPS)).astype(np.float32)
    d = (bn_bias - bn_mean * g).astype(np.float32)
    w2 = (conv_w.astype(np.float32) * g[:, None]) * WSCALE   # [M, C]
    # lhsT layout: [p, i*8+m] = w2[m, 16p+i]
    w2t = np.ascontiguousarray(
        w2.T.reshape(P, CI, M)).astype(ml_dtypes.float8_e3m4).reshape(P, CI * M)
    dvec = np.zeros((P, 1), np.float32)
    gmat3 = np.zeros((P, P), np.float32)
    for b in range(BL):
        dvec[32 * b:32 * b + M, 0] = d
        gmat3[32 * b:32 * b + M, 32 * b:32 * b + 32] = 1.0
    ident = np.tile(np.eye(32, dtype=np.float32), (4, 1)).astype(ml_dtypes.bfloat16)
    return w2t, dvec, gmat3, ident


def _core_inputs(features, conv_w, bn_scale, bn_bias, bn_mean, bn_var):
    """Full inputs -> list of per-core input dicts."""
    feats = np.asarray(features, np.float32).reshape(B, C, HW)
    w2t, dvec, gmat3, ident = _host_prep(
        np.asarray(conv_w, np.float32), np.asarray(bn_scale, np.float32),
        np.asarray(bn_bias, np.float32), np.asarray(bn_mean, np.float32),
        np.asarray(bn_var, np.float32))
    fbf = feats.astype(ml_dtypes.bfloat16)
    f8 = (feats * FSCALE).astype(ml_dtypes.float8_e3m4)
    in_maps = []
    for i in range(NCORES):
        sh8 = f8[BL * i:BL * (i + 1)]                        # [4, C, 288] fp8
        fc = np.ascontiguousarray(sh8.reshape(BL, P, CI * HW))
        shT = np.ascontiguousarray(
            fbf[BL * i:BL * (i + 1)].transpose(0, 2, 1))     # [4, 288, C] bf16
        ft = np.empty((NT, P, C), dtype=shT.dtype)
        slot = {t: i for i, t in enumerate(TORD)}
        for b in range(BL):
            ft[slot[2 * b]] = shT[b, 0:P]
            ft[slot[2 * b + 1]] = shT[b, P:2 * P]
            ft[slot[8], 32 * b:32 * b + 32] = shT[b, 256:HW]
        in_maps.append({"fc": fc, "ft": ft, "w2t": w2t, "dvec": dvec,
                        "gmat3": gmat3, "ident": ident})
    return in_maps


def kernel(features, conv_w, bn_scale, bn_bias, bn_mean, bn_var, **_kw):
    from concourse.bass_utils import run_bass_kernel_spmd

    if "nc" not in _CACHE:
        _CACHE["nc"] = _build_program()
    nc = _CACHE["nc"]

    in_maps = _core_inputs(features, conv_w, bn_scale, bn_bias, bn_mean, bn_var)
    res = run_bass_kernel_spmd(nc, in_maps, core_ids=list(range(NCORES)),
                               **_CACHE.get("run_kwargs", {}))
    _CACHE["last_results"] = res
    out = np.concatenate(
        [res.results[i]["out"].reshape(BL, M * C) for i in range(NCORES)],
        axis=0)
    return np.ascontiguousarray(out.reshape(B, M * C, 1, 1).astype(np.float32))


# revision 19
# speedup vs baseline: 1.0497x; 1.0497x over previous
"""Trainium2 Bass kernel for nn_BilinearAttnPool (B=32, C=2048, H=24, W=12, M=8).

Math notes (why this is exact enough):
  reference: attn = relu(BN(conv1x1(f)))  (attn >= 0)
             x = clip(f * attn, min=1e-6) ** 3 ; pooled = mean_hw(x) ** (1/3)
  Since attn >= 0:  clip(f*attn, eps) = attn * relu(f)  except where f*attn <= eps,
  where the reference contributes eps^3 = 1e-18 (utterly negligible).  Therefore
      pooled(b,m,c)^3 ∝ S(b,m,c) = sum_hw attn(b,m,hw)^3 * relu(f(b,c,hw))^3  -- a matmul.
  The 1/HW mean factor is a global scale that cancels in the final L2 normalize,
  as does any global constant through the ^(1/3) and sign-sqrt (monotone powers).
  pooled >= 0 always => sign-sqrt == sqrt;  z = S^(1/6) = exp(ln(S)/6), and the
  final L2 normalize folds into the exp bias:
      out = exp(ln(S)/6 - ln(n)/2),   n = sum_{m,c} S^(1/3)   (per sample).

Layout notes (v3):
  The host supplies the features TWICE (less total HBM bytes than one fp32 copy):
    fc[b, p, i, hw] = 8 * f[b, 16p+i, hw]  in fp8 e3m4 -- conv matmul rhs; the
       conv weights are pre-scaled x64 and the 1/128 is folded into the
       BN+relu activation's input scale.
    ft tiles [9, 128, C] bf16             -- hw-on-partitions, pooling side:
       tile t=2b+k (k=0,1): partition p = pixel 128k+p of sample b
       tile 8: partition 32b+j = tail pixel 256+j of sample b (packed 4x32)
  relu(f)^3 is computed tile-wise straight in the [hw, c] layout (ACT square,
  DVE max, DVE mul) and feeds the pooled matmul rhs directly with natural
  c-order columns.  attn^3 is transposed with tiny PE transposes; the hw-tail
  transpose lands at partition offset 32b via tile_position=(32b,32b) so the
  K=32 tail matmuls of all 4 samples pack into disjoint 32x32 array tiles.
  Dummy matmuls into a spare PSUM bank warm the PE HAM clock gate during the
  initial load phase.  Const loads ride SWDGE (gpsimd); feature loads are
  split across both HWDGE issuers (sync + scalar) to halve issue serialization.

Sharding: pure data parallel, batch 32 -> 8 cores x 4 samples.
"""

import numpy as np
import ml_dtypes

B, C, H, W, M = 32, 2048, 24, 12, 8
NCORES = 8
BL = B // NCORES          # 4 samples per core
HW = H * W                # 288
P = 128
CI = C // P               # 16 channels per partition
NT = 2 * BL + 1           # 9 hw-layout tiles
NWARM = 30                # PE HAM warm-up dummy matmuls
TORD = [0, 1, 2, 3, 8, 4, 5, 6, 7]   # ft DRAM arrival order
FSCALE = 2.0              # fp8 feature pre-scale
WSCALE = 64.0             # fp8 conv-weight pre-scale
BN_EPS = 1e-3

_CACHE = {}


def _build_program():
    import concourse.bass as bass
    import concourse.tile as tile
    import concourse.mybir as mybir
    from concourse import bacc

    # Pin every ACT function to the one table set that contains all of
    # Square/Relu/Ln/Exp/Copy, so the whole kernel does a single
    # ACT_TABLE_LOAD instead of ping-ponging between sets (~1.3us each).
    import concourse.bacc as bacc_mod
    _orig_tables = bacc_mod.get_activation_tables

    def _pinned_tables(arch):
        tabs = dict(_orig_tables(arch))
        if "natural_log_exp_and_others" in tabs:
            for k in tabs:
                if k != "natural_log_exp_and_others":
                    tabs[k] = set()
        return tabs

    bacc_mod.get_activation_tables = _pinned_tables
    try:
        nc = _build_inner(bacc_mod, tile, mybir)
    finally:
        bacc_mod.get_activation_tables = _orig_tables
    return nc


def _build_inner(bacc, tile, mybir):
    dt = mybir.dt
    AF = mybir.ActivationFunctionType

    nc = bacc.Bacc("TRN2", target_bir_lowering=False, debug=False,
                   num_devices=NCORES)

    fc_d = nc.declare_dram_parameter("fc", [BL, P, CI * HW], dt.float8e3, isOutput=False)
    ft_d = nc.declare_dram_parameter("ft", [NT, P, C], dt.bfloat16, isOutput=False)
    w2t_d = nc.declare_dram_parameter("w2t", [P, CI * M], dt.float8e3, isOutput=False)
    dvec_d = nc.declare_dram_parameter("dvec", [P, 1], dt.float32, isOutput=False)
    gmat3_d = nc.declare_dram_parameter("gmat3", [P, P], dt.float32, isOutput=False)
    ident_d = nc.declare_dram_parameter("ident", [P, 32], dt.bfloat16, isOutput=False)
    out_d = nc.declare_dram_parameter("out", [BL, M, C], dt.float32, isOutput=True)

    with tile.TileContext(nc) as tc:
        with (
            tc.tile_pool(name="const", bufs=1) as cpool,
            tc.tile_pool(name="persist", bufs=1) as perst,
            tc.tile_pool(name="ftp", bufs=1) as ftp,
            tc.tile_pool(name="fcp", bufs=1) as fcp,
            tc.tile_pool(name="sqp", bufs=3) as sqp,
            tc.tile_pool(name="rlp", bufs=3) as rlp,
            tc.tile_pool(name="f3p", bufs=1) as f3p,
            tc.tile_pool(name="psa", bufs=1, space="PSUM") as psa_pool,
            tc.tile_pool(name="psp", bufs=1, space="PSUM") as psp_pool,
            tc.tile_pool(name="psn", bufs=1, space="PSUM") as psn_pool,
        ):
            w2t = cpool.tile([P, CI * M], dt.float8e3)
            dvec = cpool.tile([P, 1], dt.float32)
            gmat3 = cpool.tile([P, P], dt.float32)
            ident = cpool.tile([P, 32], dt.bfloat16)
            dum = cpool.tile([P, 512], dt.bfloat16)
            nc.sync.dma_start(w2t[:], w2t_d.ap())
            nc.sync.dma_start(dvec[:], dvec_d.ap())
            nc.sync.dma_start(ident[:], ident_d.ap())
            nc.sync.dma_start(gmat3[:], gmat3_d.ap())

            ftb = [None] * NT
            fcb = [None] * BL

            def load_ft_group(slots):
                # one SWDGE DMA covering consecutive DRAM ft slots
                i0 = min(slots)
                grp = ftp.tile([P, len(slots), C], dt.bfloat16,
                               name=f"ftg{i0}", tag=f"ftg{i0}")
                for k, i in enumerate(slots):
                    ftb[TORD[i]] = grp[:, k, :]
                nc.gpsimd.dma_start(
                    grp[:],
                    ft_d.ap()[i0:i0 + len(slots)].rearrange("t p c -> p t c"))

            def load_fc(b0, n):
                grp = fcp.tile([P, n, CI, HW], dt.float8e3, name=f"fcg{b0}",
                               tag=f"fcg{b0}")
                for k in range(n):
                    fcb[b0 + k] = grp[:, k, :, :]
                nc.gpsimd.dma_start(
                    grp[:],
                    fc_d.ap()[b0:b0 + n].rearrange("b p (i hw) -> p b i hw",
                                                   i=CI))

            # all feature bytes ride SWDGE (Q0) -- the empirically fastest
            # solo ring -- fine groups during the ring ramp, then perfectly
            # alternating fc/ft groups so no engine sees bunched arrivals
            load_fc(0, 1)
            load_ft_group([0])
            load_ft_group([1])
            load_fc(1, 1)
            load_ft_group([2, 3])
            load_fc(2, 1)
            load_ft_group([4])
            load_ft_group([5, 6])
            load_fc(3, 1)
            load_ft_group([7, 8])

            attn = perst.tile([P, HW], dt.bfloat16)
            sqt = perst.tile([P, HW], dt.bfloat16)
            a3 = perst.tile([P, HW], dt.bfloat16)
            a3t = [perst.tile([P, 2, 32], dt.bfloat16, name=f"a3t{b}",
                              tag=f"a3t{b}") for b in range(BL)]
            a3tail = perst.tile([P, 32], dt.bfloat16)

            psA = psa_pool.tile([P, HW], dt.float32)
            psP = psp_pool.tile([P, C], dt.float32)
            psT3 = psn_pool.tile([P, 3, 32], dt.bfloat16, tag="pst")
            psT = psT3[:, 0:2, :]
            psTt = psT3[:, 2, :]
            psB4 = psn_pool.tile([P, BL], dt.float32, tag="psb4")
            psD = psn_pool.tile([P, P], dt.float32, tag="psdum")

            nc.vector.memset(dum[:], 0.0)

            # HAM warm-up: cheap dummies flip the PE clock gate to 2.4 GHz,
            # then 16 now-warm zero-matmuls cover all of psP (keeps ln()
            # finite on unused rows, replacing a 1.8us DVE memset)
            for _ in range(NWARM):
                nc.tensor.matmul(psD[0:M, 0:P], dum[:, 0:M], dum[:, 0:P],
                                 start=True, stop=True, skip_group_check=True)
            for jj in range(4):
                for rr in range(4):
                    nc.tensor.matmul(
                        psP[32 * rr:32 * rr + 32, 512 * jj:512 * (jj + 1)],
                        dum[:, 0:32], dum[:],
                        start=True, stop=True,
                        tile_position=(0, 32 * rr),
                        skip_group_check=True)

            f3 = [None] * NT

            def elementwise(t, sq_dve=False, rl_gps=False):
                # f3 = relu(f)^3 = f^2 * max(f, 0), all bf16 [128, 2048]
                sq = sqp.tile([P, C], dt.bfloat16)
                rl = rlp.tile([P, C], dt.bfloat16)
                f3[t] = f3p.tile([P, C], dt.bfloat16, name=f"f3_{t}", tag=f"f3_{t}")
                if sq_dve:
                    nc.vector.tensor_mul(sq[:], ftb[t][:], ftb[t][:])
                else:
                    nc.scalar.activation(sq[:], ftb[t][:], AF.Square)
                if rl_gps:
                    nc.gpsimd.tensor_scalar_max(rl[:], ftb[t][:], 0.0)
                else:
                    nc.vector.tensor_scalar_max(rl[:], ftb[t][:], 0.0)
                nc.vector.tensor_mul(f3[t][:], sq[:], rl[:])

            def elementwise67():
                # the last-arriving pair (tiles 6, 7) lands together: split
                # the two squares across ACT/DVE and order the DVE stream so
                # neither mul head-of-line blocks the other
                sq6 = sqp.tile([P, C], dt.bfloat16)
                sq7 = sqp.tile([P, C], dt.bfloat16)
                rl6 = rlp.tile([P, C], dt.bfloat16)
                rl7 = rlp.tile([P, C], dt.bfloat16)
                f3[6] = f3p.tile([P, C], dt.bfloat16, name="f3_6", tag="f3_6")
                f3[7] = f3p.tile([P, C], dt.bfloat16, name="f3_7", tag="f3_7")
                nc.scalar.activation(sq6[:], ftb[6][:], AF.Square)
                nc.vector.tensor_scalar_max(rl6[:], ftb[6][:], 0.0)
                nc.vector.tensor_scalar_max(rl7[:], ftb[7][:], 0.0)
                nc.vector.tensor_mul(sq7[:], ftb[7][:], ftb[7][:])
                nc.vector.tensor_mul(f3[6][:], sq6[:], rl6[:])
                nc.vector.tensor_mul(f3[7][:], sq7[:], rl7[:])

            def conv(b):
                for i in range(CI):
                    nc.tensor.matmul(
                        psA[32 * b:32 * b + M, :],
                        w2t[:, M * i:M * (i + 1)],
                        fcb[b][:, i, :],
                        start=(i == 0), stop=(i == CI - 1),
                        tile_position=(0, 32 * b),
                        skip_group_check=True,
                    )

            def attn_cube(b):
                # attn = relu(conv/512 + d); a3 = attn^3; transpose to [hw, m]
                rs = slice(32 * b, 32 * b + M)
                ts = slice(32 * b, 32 * b + 32)
                nc.scalar.activation(attn[rs, :], psA[rs, :], AF.Relu,
                                     bias=dvec[rs, :],
                                     scale=1.0 / (FSCALE * WSCALE))
                nc.scalar.activation(sqt[rs, :], attn[rs, :], AF.Square)
                nc.vector.tensor_mul(a3[rs, :], sqt[rs, :], attn[rs, :])
                for k in range(2):
                    nc.tensor.transpose(
                        psT[:, k, :], a3[ts, P * k:P * (k + 1)], ident[ts, :],
                        tile_position=(32 * b, 0))
                nc.tensor.transpose(
                    psTt[ts], a3[ts, 256:HW], ident[ts, :],
                    tile_position=(32 * b, 32 * b))
                nc.vector.tensor_copy(a3t[b][:], psT[:])
                nc.vector.tensor_copy(a3tail[ts, :], psTt[ts])

            def pooled(b):
                # psP[32b+m, c] = sum_hw a3t[hw, m] * f3[hw, c]
                rs = slice(32 * b, 32 * b + M)
                ts = slice(32 * b, 32 * b + 32)
                def mm_k(k, j, start):
                    cs = slice(512 * j, 512 * (j + 1))
                    rhs = f3[2 * b + k][:, cs]
                    nc.tensor.matmul(
                        psP[rs, cs], a3t[b][:, k, 0:M], rhs,
                        start=start, stop=False,
                        tile_position=(0, 32 * b),
                        skip_group_check=True)
                def mm_tail(j):
                    cs = slice(512 * j, 512 * (j + 1))
                    nc.tensor.matmul(
                        psP[rs, cs], a3tail[ts, 0:M], f3[8][ts, cs],
                        start=False, stop=True,
                        tile_position=(32 * b, 32 * b),
                        skip_group_check=True)
                if b < BL - 1:
                    for k in range(2):
                        for j in range(4):
                            mm_k(k, j, start=(k == 0))
                else:
                    # last sample: finish psP chunk-by-chunk so the Ln/Exp
                    # post chunks overlap the remaining matmuls
                    for j in range(4):
                        mm_k(0, j, start=True)
                        mm_k(1, j, start=False)
                        mm_tail(j)

            def pooled_tails(b):
                rs = slice(32 * b, 32 * b + M)
                ts = slice(32 * b, 32 * b + 32)
                for j in range(4):
                    cs = slice(512 * j, 512 * (j + 1))
                    nc.tensor.matmul(
                        psP[rs, cs], a3tail[ts, 0:M], f3[8][ts, cs],
                        start=False, stop=True,
                        tile_position=(32 * b, 32 * b),
                        skip_group_check=True)

            # software-pipelined emission: conv one sample ahead of the
            # attn/pooled chain so the PE stream never head-of-line blocks
            conv(0)
            elementwise(0)
            elementwise(1)
            conv(1)
            attn_cube(0)
            pooled(0)
            elementwise(2)
            elementwise(8)
            elementwise(3)
            conv(2)
            attn_cube(1)
            pooled(1)
            elementwise(4, sq_dve=True)
            elementwise(5, sq_dve=True)
            conv(3)
            attn_cube(2)
            pooled(2)
            pooled_tails(0)
            pooled_tails(1)
            pooled_tails(2)
            attn_cube(3)
            elementwise67()
            pooled(3)

            # post: z = S^(1/6), n = sum_c S^(1/3), out = exp(ln(S)/6 - ln(n)/2)
            # Ln and the S^(1/3) row-accumulate run per 512-col chunk so they
            # overlap the final sample's pooled matmuls.
            lns = perst.tile([P, C], dt.float32)
            wsb = perst.tile([P, C], dt.bfloat16)
            part = perst.tile([P, 2], dt.float32)
            nsum = perst.tile([P, 1], dt.float32)
            lnn = perst.tile([P, 1], dt.float32)
            nbias = perst.tile([P, 1], dt.float32)
            fm = perst.tile([P, C], dt.float32)

            for j in range(2):
                cs = slice(1024 * j, 1024 * (j + 1))
                nc.scalar.activation(lns[:, cs], psP[:, cs], AF.Ln)
                nc.scalar.activation(wsb[:, cs], lns[:, cs], AF.Exp,
                                     scale=1.0 / 3.0,
                                     accum_out=part[:, j:j + 1])
            nc.tensor.matmul(psB4[:, 0:2], gmat3[:], part[:])  # group sums
            scr2 = perst.tile([P, 2], dt.float32)
            nc.scalar.activation(scr2[:], psB4[:, 0:2], AF.Copy,
                                 accum_out=nsum[:])
            nc.scalar.activation(lnn[:], nsum[:], AF.Ln)
            nc.vector.tensor_scalar_mul(nbias[:], lnn[:], -0.5)
            nc.scalar.activation(fm[:], lns[:], AF.Exp, scale=1.0 / 6.0,
                                 bias=nbias[:])

            nc.sync.dma_start(out_d.ap()[0], fm[0:M, :])
            nc.scalar.dma_start(out_d.ap()[1], fm[32:32 + M, :])
            nc.sync.dma_start(out_d.ap()[2], fm[64:64 + M, :])
            nc.scalar.dma_start(out_d.ap()[3], fm[96:96 + M, :])

    nc.compile()
    return nc


def _host_prep(conv_w, bn_scale, bn_bias, bn_mean, bn_var):
    g = (bn_scale / np.sqrt(bn_var + BN_EPS)).astype(np.float32)
    d = (bn_bias - bn_mean * g).astype(np.float32)
    w2 = (conv_w.astype(np.float32) * g[:, None]) * WSCALE   # [M, C]
    # lhsT layout: [p, i*8+m] = w2[m, 16p+i]
    w2t = np.ascontiguousarray(
        w2.T.reshape(P, CI, M)).astype(ml_dtypes.float8_e3m4).reshape(P, CI * M)
    dvec = np.zeros((P, 1), np.float32)
    gmat3 = np.zeros((P, P), np.float32)
    for b in range(BL):
        dvec[32 * b:32 * b + M, 0] = d
        gmat3[32 * b:32 * b + M, 32 * b:32 * b + 32] = 1.0
    ident = np.tile(np.eye(32, dtype=np.float32), (4, 1)).astype(ml_dtypes.bfloat16)
    return w2t, dvec, gmat3, ident


def _core_inputs(features, conv_w, bn_scale, bn_bias, bn_mean, bn_var):
    """Full inputs -> list of per-core input dicts."""
    feats = np.asarray(features, np.float32).reshape(B, C, HW)
    w2t, dvec, gmat3, ident = _host_prep(
        np.asarray(conv_w, np.float32), np.asarray(bn_scale, np.float32),
        np.asarray(bn_bias, np.float32), np.asarray(bn_mean, np.float32),
        np.asarray(bn_var, np.float32))
    fbf = feats.astype(ml_dtypes.bfloat16)
    f8 = (feats * FSCALE).astype(ml_dtypes.float8_e3m4)
    in_maps = []
    for i in range(NCORES):
        sh8 = f8[BL * i:BL * (i + 1)]                        # [4, C, 288] fp8
        fc = np.ascontiguousarray(sh8.reshape(BL, P, CI * HW))
        shT = np.ascontiguousarray(
            fbf[BL * i:BL * (i + 1)].transpose(0, 2, 1))     # [4, 288, C] bf16
        ft = np.empty((NT, P, C), dtype=shT.dtype)
        slot = {t: i for i, t in enumerate(TORD)}
        for b in range(BL):
            ft[slot[2 * b]] = shT[b, 0:P]
            ft[slot[2 * b + 1]] = shT[b, P:2 * P]
            ft[slot[8], 32 * b:32 * b + 32] = shT[b, 256:HW]
        in_maps.append({"fc": fc, "ft": ft, "w2t": w2t, "dvec": dvec,
                        "gmat3": gmat3, "ident": ident})
    return in_maps


def kernel(features, conv_w, bn_scale, bn_bias, bn_mean, bn_var, **_kw):
    from concourse.bass_utils import run_bass_kernel_spmd

    if "nc" not in _CACHE:
        _CACHE["nc"] = _build_program()
    nc = _CACHE["nc"]

    in_maps = _core_inputs(features, conv_w, bn_scale, bn_bias, bn_mean, bn_var)
    res = run_bass_kernel_spmd(nc, in_maps, core_ids=list(range(NCORES)),
                               **_CACHE.get("run_kwargs", {}))
    _CACHE["last_results"] = res
    out = np.concatenate(
        [res.results[i]["out"].reshape(BL, M * C) for i in range(NCORES)],
        axis=0)
    return np.ascontiguousarray(out.reshape(B, M * C, 1, 1).astype(np.float32))
